# revision 5
# baseline (speedup 1.0000x reference)
"""Trainium2 Bass kernel for the contrastive loss problem.

Strategy (8 NeuronCores, SPMD, symmetric-half E):
  - Host normalizes features in f32 (exact norms), casts to bf16 and
    pre-transposes: zT [D=128, N] so the device does no transposes.
  - Global row-chunk g (of 64) is the anchor of exactly one core
    (core c anchors local chunks 0..7 = global 8c..8c+7).  Anchor a
    computes E blocks against column chunks a+1..a+32 only (each
    unordered chunk pair lands on exactly one anchor, except distance
    32 which both sides compute and count row-wise only).
  - Row sums come free from the ScalarE activation accumulator.
    Mirror (column) sums: DVE accumulates the fp16 E slabs into
    col_acc[128, 38*128]; the host does the final partition sum.
  - Diagonal blocks are recomputed in fp32 (pos terms need precision),
    masked-reduced on DVE for S.
  - Host combines: T = own rowsums + diag + mirror colsums,
    pos = S - e^(1/tau), neg = T - S, loss = mean(log(neg) - log(pos)).
"""

import sys

import numpy as np

sys.path.insert(0, "/opt/trn_rl_repo")

N, D = 8192, 128
NCORES = 8
RPC = N // NCORES  # rows per core (1024)
CHUNKS = N // 128  # 64 chunks
ACH = 8  # anchor chunks per core
NCOL = 32  # column chunks per anchor (distances 1..32)
MIRC = 38  # mirror chunks with col-acc (distances 1..31 -> local 1..38)
ZTC = 40  # zt column chunks a core needs (0..39)
TAU = 0.5
EPS = 1e-8

_PROGRAM = None
_COMPILE_PATCHED = False


def _patch_compile():
    """This container's walrus build rejects two instructions that the Tile
    framework emits in its kernel tail: a Drain carrying more than one sem
    wait ("Too many sync wait commands") and the EVENT_SEMAPHORE_RANGE_CLEAR
    ISA instruction ("ISA wrong length").  Rewrite the BIR before walrus sees
    it: split multi-wait Drains into chains of single-wait Drains, and drop
    the range-clear (sems are left dirty, so one NEFF load supports a single
    execution -- kernel() is called once per process, which is our usage)."""
    global _COMPILE_PATCHED
    if _COMPILE_PATCHED:
        return
    import orjson

    import concourse.bass2jax as bass2jax
    import concourse.bass_utils as bass_utils

    orig = bass_utils.compile_bir_kernel

    def patched(bir_json, tmpdir, neff_name="file.neff"):
        bir = orjson.loads(bir_json)
        for fn in bir.get("functions", []):
            for bb in fn.get("blocks", []):
                new_insts = []
                for ins in bb.get("instructions", []):
                    if (
                        ins.get("opcode") == "ISA"
                        and ins.get("isa_opcode") == 176
                    ):
                        continue  # EVENT_SEMAPHORE_RANGE_CLEAR
                    sync = ins.get("sync_info")
                    if sync and len(sync.get("on_wait") or []) > 1:
                        waits = sync["on_wait"]
                        for k, w in enumerate(waits[:-1]):
                            pre = {
                                "engine": ins["engine"],
                                "name": f"{ins['name']}_w{k}",
                                "opcode": "Drain",
                                "ins": [],
                                "outs": [],
                                "sync_info": {"on_update": [], "on_wait": [w]},
                            }
                            if "debug" in ins:
                                pre["debug"] = ins["debug"]
                            new_insts.append(pre)
                        sync["on_wait"] = [waits[-1]]
                    new_insts.append(ins)
                bb["instructions"] = new_insts
        return orig(orjson.dumps(bir), tmpdir, neff_name=neff_name)

    bass_utils.compile_bir_kernel = patched
    bass2jax.compile_bir_kernel = patched
    _COMPILE_PATCHED = True


def _build_program():
    import concourse.bass as bass
    import concourse.mybir as mybir
    import concourse.tile as tile

    f32 = mybir.dt.float32
    f16 = mybir.dt.float16
    bf16 = mybir.dt.bfloat16
    AF = mybir.ActivationFunctionType
    AX = mybir.AxisListType

    nc = bass.Bass("TRN2", target_bir_lowering=False, debug=False)

    zt_d = nc.dram_tensor("zt", [128, ZTC * 128], bf16, kind="ExternalInput")
    zt32_d = nc.dram_tensor("zt32", [128, RPC], f32, kind="ExternalInput")
    mask_d = nc.dram_tensor("mask", [128, 128], f32, kind="ExternalInput")
    pout_d = nc.dram_tensor("pout", [128, 24], f32, kind="ExternalOutput")
    cacc_d = nc.dram_tensor("cacc", [128, MIRC * 128], f16, kind="ExternalOutput")

    with tile.TileContext(nc) as tc:
        with (
            tc.tile_pool(name="singles", bufs=1) as singles,
            tc.tile_pool(name="scratch", bufs=2) as scratch,
            tc.tile_pool(name="es", bufs=3) as es,
            tc.tile_pool(name="pm", bufs=2, space="PSUM") as pm,
        ):
            zt_sb = singles.tile([128, ZTC * 128], bf16)
            zt32_sb = singles.tile([128, RPC], f32)
            mask_sb = singles.tile([128, 128], f32)
            col_acc = singles.tile([128, MIRC * 128], f16)
            ediag = singles.tile([128, RPC], f32)
            tacc = singles.tile([128, 16], f32)
            pout_sb = singles.tile([128, 24], f32)

            # ---- input DMAs: zt32+mask first (diag path), zt spread wide
            for i in range(4):
                c0, c1 = i * 256, (i + 1) * 256
                eng = nc.scalar if i % 2 == 0 else nc.sync
                eng.dma_start(zt32_sb[:, c0:c1], zt32_d.ap()[:, c0:c1])
            nc.scalar.dma_start(mask_sb[:], mask_d.ap())
            for i in range(10):
                c0, c1 = i * 512, (i + 1) * 512
                eng = nc.sync if i % 2 == 0 else nc.scalar
                eng.dma_start(zt_sb[:, c0:c1], zt_d.ap()[:, c0:c1])

            # ---- diagonal blocks in fp32 ----
            pdiag = pm.tile([128, 2048], f32, tag="pm")
            for A in range(ACH):
                zc = zt32_sb[:, A * 128 : (A + 1) * 128]
                nc.tensor.matmul(
                    pdiag[:, A * 128 : (A + 1) * 128], zc, zc,
                    start=True, stop=True,
                )
            nc.scalar.activation(
                out=ediag[:, 0:1024], in_=pdiag[:, 0:1024],
                func=AF.Exp, scale=2.0,
            )
            # tdf: per-chunk row sums of ediag
            nc.vector.reduce_sum(
                out=pout_sb[:, 8:16],
                in_=ediag.rearrange("p (a q) -> p a q", a=ACH),
                axis=AX.X,
            )
            # S: masked per-chunk sums
            for A in range(ACH):
                mtmp = scratch.tile([128, 128], f32, tag="mt")
                nc.vector.tensor_mul(
                    mtmp[:], ediag[:, A * 128 : (A + 1) * 128], mask_sb[:]
                )
                nc.vector.reduce_sum(
                    out=pout_sb[:, 16 + A : 17 + A], in_=mtmp[:], axis=AX.X
                )

            # ---- anchors: E slabs over column chunks a+1..a+32 ----
            for a in range(ACH):
                lhsT = zt_sb[:, a * 128 : (a + 1) * 128]
                for s in range(2):
                    base = (a + 1) * 128 + s * 2048
                    pt = pm.tile([128, 2048], f32, tag="pm")
                    for m in range(4):
                        nc.tensor.matmul(
                            pt[:, m * 512 : (m + 1) * 512],
                            lhsT,
                            zt_sb[:, base + m * 512 : base + (m + 1) * 512],
                            start=True, stop=True,
                        )
                    et = es.tile([128, 2048], f16)
                    nc.scalar.activation(
                        out=et[:], in_=pt[:], func=AF.Exp, scale=2.0,
                        accum_out=tacc[:, a * 2 + s : a * 2 + s + 1],
                    )
                    # mirror accumulation (exclude distance-32 chunk).
                    # First touch of each col_acc chunk is a copy: anchor 0
                    # touches chunks 1..31 first; anchor a>=1 first-touches
                    # only chunk a+31 (last 128 cols of its s==1 slab).
                    w = 2048 if s == 0 else 1920
                    off = a * 128 + s * 2048  # col_acc col 0 == local chunk 1
                    if a == 0:
                        nc.vector.tensor_copy(
                            col_acc[:, off : off + w], et[:, 0:w]
                        )
                    elif s == 0:
                        nc.vector.tensor_add(
                            col_acc[:, off : off + w],
                            et[:, 0:w],
                            col_acc[:, off : off + w],
                        )
                    else:
                        nc.vector.tensor_add(
                            col_acc[:, off : off + 1792],
                            et[:, 0:1792],
                            col_acc[:, off : off + 1792],
                        )
                        nc.vector.tensor_copy(
                            col_acc[:, off + 1792 : off + 1920],
                            et[:, 1792:1920],
                        )

                    # chase-DMA cacc pieces as their col_acc regions
                    # finalize (chunk j's last writer is anchor min(7,j-1))
                    if a == 6 and s == 1:
                        pieces = [(0, 512)]
                    elif a == 7 and s == 0:
                        pieces = [(512 * p, 512 * (p + 1)) for p in (1, 2, 3, 4)]
                    elif a == 7 and s == 1:
                        pieces = [
                            (2560, 3072), (3072, 3584), (3584, 4096),
                            (4096, 4608), (4608, 4864),
                        ]
                    else:
                        pieces = []
                    for k, (p0, p1) in enumerate(pieces):
                        eng = nc.sync if k % 2 == 0 else nc.scalar
                        eng.dma_start(
                            cacc_d.ap()[:, p0:p1], col_acc[:, p0:p1]
                        )

            # T_main = sum of the two per-anchor slab accumulators
            nc.vector.reduce_sum(
                out=pout_sb[:, 0:8],
                in_=tacc.rearrange("p (a s) -> p a s", a=ACH),
                axis=AX.X,
            )

            # ---- outputs ----
            nc.sync.dma_start(pout_d.ap(), pout_sb[:])

    return nc


def _get_program():
    global _PROGRAM
    if _PROGRAM is None:
        _PROGRAM = _build_program()
    return _PROGRAM


def _group_ids(num_crops):
    ids = np.repeat(np.arange(num_crops.shape[0], dtype=np.int64), num_crops)
    if ids.shape[0] >= N:
        return ids[:N]
    return np.pad(ids, (0, N - ids.shape[0]), mode="edge")


def _build_mask(num_crops):
    """[128,128] same-group mask, valid when the group pattern repeats
    every 128 rows and no group straddles a 128-row boundary."""
    ids = _group_ids(num_crops)
    pat = ids.reshape(CHUNKS, 128)
    local = pat - pat[:, :1]
    if not (local == local[0]).all():
        return None
    if (pat[1:, 0] <= pat[:-1, -1]).any():
        return None
    return (local[0][:, None] == local[0][None, :]).astype(np.float32)


def _prep(feat):
    """Host prep: exact f32 normalize, bf16 cast, transpose."""
    import ml_dtypes

    nrm = np.maximum(np.sqrt((feat.astype(np.float64) ** 2).sum(-1)), EPS)
    z32 = (feat / nrm[:, None]).astype(np.float32)
    zbfT = np.ascontiguousarray(z32.astype(ml_dtypes.bfloat16).T)  # [128, N]
    z32T = np.ascontiguousarray(z32.T)  # [128, N]
    return zbfT, z32T


def _make_inmaps(feat, mask):
    zbfT, z32T = _prep(feat)
    in_maps = []
    for c in range(NCORES):
        zt = np.ascontiguousarray(
            np.roll(zbfT, -RPC * c, axis=1)[:, : ZTC * 128]
        )
        zt32 = np.ascontiguousarray(z32T[:, RPC * c : RPC * (c + 1)])
        in_maps.append({"zt": zt, "zt32": zt32, "mask": mask})
    return in_maps


def _combine(results):
    """Host combine of per-core partials -> loss (f64)."""
    T = np.zeros(N, np.float64)
    S = np.zeros(N, np.float64)
    for c in range(NCORES):
        r = results[c]
        pout = r["pout"].astype(np.float64)  # [128, 24]
        tmain, tdf, s = pout[:, 0:8], pout[:, 8:16], pout[:, 16:24]
        for a in range(ACH):
            g = 8 * c + a
            rows = slice(g * 128, (g + 1) * 128)
            T[rows] += tmain[:, a] + tdf[:, a]
            S[rows] = s[:, a]
        cs = r["cacc"].astype(np.float64).sum(axis=0).reshape(MIRC, 128)
        for j in range(1, MIRC + 1):
            g = (8 * c + j) % CHUNKS
            T[g * 128 : (g + 1) * 128] += cs[j - 1]
    pos = S - np.exp(1.0 / TAU)
    neg = T - S
    return np.asarray(np.mean(np.log(neg) - np.log(pos)), dtype=np.float32)


def _numpy_fallback(feat, num_crops):
    ids = _group_ids(num_crops)
    nrm = np.maximum(np.sqrt((feat.astype(np.float64) ** 2).sum(-1)), EPS)
    z = feat / nrm[:, None].astype(np.float32)
    T = np.empty(N, np.float64)
    S = np.empty(N, np.float64)
    for r0 in range(0, N, 512):
        E = np.exp((z[r0 : r0 + 512] @ z.T) / TAU).astype(np.float64)
        same = ids[r0 : r0 + 512, None] == ids[None, :]
        T[r0 : r0 + 512] = E.sum(1)
        S[r0 : r0 + 512] = np.where(same, E, 0.0).sum(1)
    pos = S - np.exp(1.0 / TAU)
    neg = T - S
    return np.asarray(np.mean(np.log(neg) - np.log(pos)), dtype=np.float32)


def kernel(features, num_crops):
    feat = np.ascontiguousarray(np.asarray(features, dtype=np.float32))
    ncr = np.asarray(num_crops)
    mask = _build_mask(ncr)
    if mask is None:
        return _numpy_fallback(feat, ncr)

    _patch_compile()
    from concourse.bass_utils import run_bass_kernel_spmd

    nc = _get_program()
    in_maps = _make_inmaps(feat, mask)
    res = run_bass_kernel_spmd(nc, in_maps, core_ids=list(range(NCORES)))
    return _combine(res.results)


# revision 8
# speedup vs baseline: 1.0556x; 1.0556x over previous
"""Trainium2 Bass kernel for the contrastive loss problem.

Strategy (8 NeuronCores, SPMD, symmetric-half E):
  - Host normalizes features in f32 (exact norms), casts to bf16 and
    pre-transposes: zT [D=128, N] so the device does no transposes.
  - Global row-chunk g (of 64) is the anchor of exactly one core
    (core c anchors local chunks 0..7 = global 8c..8c+7).  Anchor a
    computes E blocks against column chunks a+1..a+32 only (each
    unordered chunk pair lands on exactly one anchor, except distance
    32 which both sides compute and count row-wise only).
  - Row sums come free from the ScalarE activation accumulator.
    Mirror (column) sums: DVE accumulates the fp16 E slabs into
    col_acc[128, 38*128]; the host does the final partition sum.
  - Diagonal blocks are bf16 like everything else; the self-term
    exp(2*||z_bf||^2) is computed exactly on the host, so the bf16
    rounding of the remaining ~3 same-group terms averages out over
    the 8192-row mean (<<1e-3 on the loss).
  - Host combines: T = own rowsums + diag + mirror colsums,
    pos = S - selfE, neg = T - S, loss = mean(log(neg) - log(pos)).
"""

import sys

import numpy as np

sys.path.insert(0, "/opt/trn_rl_repo")

N, D = 8192, 128
NCORES = 8
RPC = N // NCORES  # rows per core (1024)
CHUNKS = N // 128  # 64 chunks
ACH = 8  # anchor chunks per core
MIRC = 38  # mirror chunks with col-acc (distances 1..31 -> local 1..38)
ZTC = 40  # zt column chunks a core needs (0..39)
TAU = 0.5
EPS = 1e-8

_PROGRAM = None
_COMPILE_PATCHED = False


def _patch_compile():
    """This container's walrus build rejects two instructions that the Tile
    framework emits in its kernel tail: a Drain carrying more than one sem
    wait ("Too many sync wait commands") and the EVENT_SEMAPHORE_RANGE_CLEAR
    ISA instruction ("ISA wrong length").  Rewrite the BIR before walrus sees
    it: split multi-wait Drains into chains of single-wait Drains, and drop
    the range-clear (sems are left dirty, so one NEFF load supports a single
    execution -- kernel() is called once per process, which is our usage)."""
    global _COMPILE_PATCHED
    if _COMPILE_PATCHED:
        return
    import orjson

    import concourse.bass2jax as bass2jax
    import concourse.bass_utils as bass_utils

    orig = bass_utils.compile_bir_kernel

    def patched(bir_json, tmpdir, neff_name="file.neff"):
        bir = orjson.loads(bir_json)
        for fn in bir.get("functions", []):
            for bb in fn.get("blocks", []):
                new_insts = []
                for ins in bb.get("instructions", []):
                    if (
                        ins.get("opcode") == "ISA"
                        and ins.get("isa_opcode") == 176
                    ):
                        continue  # EVENT_SEMAPHORE_RANGE_CLEAR
                    sync = ins.get("sync_info")
                    if sync and len(sync.get("on_wait") or []) > 1:
                        waits = sync["on_wait"]
                        for k, w in enumerate(waits[:-1]):
                            pre = {
                                "engine": ins["engine"],
                                "name": f"{ins['name']}_w{k}",
                                "opcode": "Drain",
                                "ins": [],
                                "outs": [],
                                "sync_info": {"on_update": [], "on_wait": [w]},
                            }
                            if "debug" in ins:
                                pre["debug"] = ins["debug"]
                            new_insts.append(pre)
                        sync["on_wait"] = [waits[-1]]
                    new_insts.append(ins)
                bb["instructions"] = new_insts
        return orig(orjson.dumps(bir), tmpdir, neff_name=neff_name)

    bass_utils.compile_bir_kernel = patched
    bass2jax.compile_bir_kernel = patched
    _COMPILE_PATCHED = True


def _build_program():
    import concourse.bass as bass
    import concourse.mybir as mybir
    import concourse.tile as tile

    f32 = mybir.dt.float32
    f16 = mybir.dt.float16
    bf16 = mybir.dt.bfloat16
    AF = mybir.ActivationFunctionType
    AX = mybir.AxisListType

    nc = bass.Bass("TRN2", target_bir_lowering=False, debug=False)

    zt_d = nc.dram_tensor("zt", [128, ZTC * 128], bf16, kind="ExternalInput")
    mask_d = nc.dram_tensor("mask", [128, 128], f32, kind="ExternalInput")
    pout_d = nc.dram_tensor("pout", [128, 24], f32, kind="ExternalOutput")
    cacc_d = nc.dram_tensor("cacc", [128, MIRC * 128], f16, kind="ExternalOutput")

    with tile.TileContext(nc) as tc:
        with (
            tc.tile_pool(name="singles", bufs=1) as singles,
            tc.tile_pool(name="scratch", bufs=2) as scratch,
            tc.tile_pool(name="es", bufs=3) as es,
            tc.tile_pool(name="pm", bufs=2, space="PSUM") as pm,
        ):
            zt_sb = singles.tile([128, ZTC * 128], bf16)
            mask_sb = singles.tile([128, 128], f32)
            col_acc = singles.tile([128, MIRC * 128], f16)
            ediag = singles.tile([128, RPC], f32)
            tacc = singles.tile([128, 20], f32)
            pout_sb = singles.tile([128, 24], f32)

            # ---- input DMAs: first two small pieces feed the diag path
            nc.scalar.dma_start(zt_sb[:, 0:256], zt_d.ap()[:, 0:256])
            nc.sync.dma_start(zt_sb[:, 256:512], zt_d.ap()[:, 256:512])
            nc.sync.dma_start(zt_sb[:, 512:768], zt_d.ap()[:, 512:768])
            nc.scalar.dma_start(zt_sb[:, 768:1024], zt_d.ap()[:, 768:1024])
            for i in range(8):
                c0, c1 = 1024 + i * 512, 1024 + (i + 1) * 512
                eng = nc.sync if i % 2 == 0 else nc.scalar
                eng.dma_start(zt_sb[:, c0:c1], zt_d.ap()[:, c0:c1])
            nc.sync.dma_start(mask_sb[:], mask_d.ap())

            # ---- diagonal blocks (bf16; self-term fixed on host) ----
            pdiag = pm.tile([128, 2048], f32, tag="pm")
            for A in range(ACH):
                zc = zt_sb[:, A * 128 : (A + 1) * 128]
                nc.tensor.matmul(
                    pdiag[:, A * 128 : (A + 1) * 128], zc, zc,
                    start=True, stop=True,
                )
            nc.scalar.activation(
                out=ediag[:, 0:1024], in_=pdiag[:, 0:1024],
                func=AF.Exp, scale=2.0,
            )
            # tdf: per-chunk row sums of ediag
            nc.vector.reduce_sum(
                out=pout_sb[:, 8:16],
                in_=ediag.rearrange("p (a q) -> p a q", a=ACH),
                axis=AX.X,
            )
            # S: masked per-chunk sums
            for A in range(ACH):
                mtmp = scratch.tile([128, 128], f32, tag="mt")
                nc.vector.tensor_mul(
                    mtmp[:], ediag[:, A * 128 : (A + 1) * 128], mask_sb[:]
                )
                nc.vector.reduce_sum(
                    out=pout_sb[:, 16 + A : 17 + A], in_=mtmp[:], axis=AX.X
                )

            # ---- anchors: E slabs over column chunks a+1..a+32 ----
            # anchors 0..6: two [128,2048] slabs; anchor 7: four
            # [128,1024] slabs so col_acc regions finalize piecewise and
            # the output DMAs can chase them.
            def slab(a, base, width, acc_idx, add_w, copy_w):
                lhsT = zt_sb[:, a * 128 : (a + 1) * 128]
                pt = pm.tile([128, 2048], f32, tag="pm")
                for m in range(width // 512):
                    nc.tensor.matmul(
                        pt[:, m * 512 : (m + 1) * 512],
                        lhsT,
                        zt_sb[:, base + m * 512 : base + (m + 1) * 512],
                        start=True, stop=True,
                    )
                et = es.tile([128, 2048], f16)
                nc.scalar.activation(
                    out=et[:, 0:width], in_=pt[:, 0:width],
                    func=AF.Exp, scale=2.0,
                    accum_out=tacc[:, acc_idx : acc_idx + 1],
                )
                off = base - 128  # col_acc col 0 == local chunk 1
                if a == 0:
                    nc.vector.tensor_copy(
                        col_acc[:, off : off + add_w + copy_w],
                        et[:, 0 : add_w + copy_w],
                    )
                else:
                    if add_w:
                        nc.vector.tensor_add(
                            col_acc[:, off : off + add_w],
                            et[:, 0:add_w],
                            col_acc[:, off : off + add_w],
                        )
                    if copy_w:
                        nc.vector.tensor_copy(
                            col_acc[:, off + add_w : off + add_w + copy_w],
                            et[:, add_w : add_w + copy_w],
                        )

            for a in range(7):
                # s=0: chunks a+1..a+16 (all RMW for a>0); s=1: chunks
                # a+17..a+32, of which a+31 is first-touch, a+32 excluded
                slab(a, (a + 1) * 128, 2048, 2 * a, 2048, 0)
                slab(a, (a + 1) * 128 + 2048, 2048, 2 * a + 1, 1792, 128)
                if a == 6:
                    nc.sync.dma_start(
                        cacc_d.ap()[:, 0:896], col_acc[:, 0:896]
                    )

            a = 7
            engs = [nc.sync, nc.scalar, nc.sync, nc.scalar]
            for q in range(4):
                base = (a + 1) * 128 + q * 1024
                add_w = 1024 if q < 3 else 768
                copy_w = 128 if q == 3 else 0
                slab(a, base, 1024, 14 + q, add_w, copy_w)
                # chase: this slab finalized col_acc [896+1024q, ...)
                p0 = 896 + q * 1024
                p1 = p0 + (1024 if q < 3 else 896)
                engs[q].dma_start(cacc_d.ap()[:, p0:p1], col_acc[:, p0:p1])

            # T_main = sum of the per-anchor slab accumulators
            nc.vector.reduce_sum(
                out=pout_sb[:, 0:7],
                in_=tacc[:, 0:14].rearrange("p (a s) -> p a s", a=7),
                axis=AX.X,
            )
            nc.vector.reduce_sum(
                out=pout_sb[:, 7:8],
                in_=tacc[:, 14:18].rearrange("p (a s) -> p a s", a=1),
                axis=AX.X,
            )

            # ---- outputs ----
            nc.sync.dma_start(pout_d.ap(), pout_sb[:])

    return nc


def _get_program():
    global _PROGRAM
    if _PROGRAM is None:
        _PROGRAM = _build_program()
    return _PROGRAM


def _group_ids(num_crops):
    ids = np.repeat(np.arange(num_crops.shape[0], dtype=np.int64), num_crops)
    if ids.shape[0] >= N:
        return ids[:N]
    return np.pad(ids, (0, N - ids.shape[0]), mode="edge")


def _build_mask(num_crops):
    """[128,128] same-group mask, valid when the group pattern repeats
    every 128 rows and no group straddles a 128-row boundary."""
    ids = _group_ids(num_crops)
    pat = ids.reshape(CHUNKS, 128)
    local = pat - pat[:, :1]
    if not (local == local[0]).all():
        return None
    if (pat[1:, 0] <= pat[:-1, -1]).any():
        return None
    return (local[0][:, None] == local[0][None, :]).astype(np.float32)


def _prep(feat):
    """Host prep: exact f32 normalize, bf16 cast, transpose, self-terms."""
    import ml_dtypes

    nrm = np.maximum(np.sqrt((feat.astype(np.float64) ** 2).sum(-1)), EPS)
    z32 = (feat / nrm[:, None]).astype(np.float32)
    zbf = z32.astype(ml_dtypes.bfloat16)
    zbfT = np.ascontiguousarray(zbf.T)  # [128, N]
    self_e = np.exp(2.0 * (zbf.astype(np.float64) ** 2).sum(-1))  # [N]
    return zbfT, self_e


def _make_inmaps(feat, mask):
    zbfT, self_e = _prep(feat)
    in_maps = []
    for c in range(NCORES):
        zt = np.ascontiguousarray(
            np.roll(zbfT, -RPC * c, axis=1)[:, : ZTC * 128]
        )
        in_maps.append({"zt": zt, "mask": mask})
    return in_maps, self_e


def _combine(results, self_e):
    """Host combine of per-core partials -> loss (f64)."""
    T = np.zeros(N, np.float64)
    S = np.zeros(N, np.float64)
    for c in range(NCORES):
        r = results[c]
        pout = r["pout"].astype(np.float64)  # [128, 24]
        tmain, tdf, s = pout[:, 0:8], pout[:, 8:16], pout[:, 16:24]
        for a in range(ACH):
            g = 8 * c + a
            rows = slice(g * 128, (g + 1) * 128)
            T[rows] += tmain[:, a] + tdf[:, a]
            S[rows] = s[:, a]
        cs = r["cacc"].astype(np.float64).sum(axis=0).reshape(MIRC, 128)
        for j in range(1, MIRC + 1):
            g = (8 * c + j) % CHUNKS
            T[g * 128 : (g + 1) * 128] += cs[j - 1]
    pos = S - self_e
    neg = T - S
    return np.asarray(np.mean(np.log(neg) - np.log(pos)), dtype=np.float32)


def _numpy_fallback(feat, num_crops):
    ids = _group_ids(num_crops)
    nrm = np.maximum(np.sqrt((feat.astype(np.float64) ** 2).sum(-1)), EPS)
    z = feat / nrm[:, None].astype(np.float32)
    T = np.empty(N, np.float64)
    S = np.empty(N, np.float64)
    for r0 in range(0, N, 512):
        E = np.exp((z[r0 : r0 + 512] @ z.T) / TAU).astype(np.float64)
        same = ids[r0 : r0 + 512, None] == ids[None, :]
        T[r0 : r0 + 512] = E.sum(1)
        S[r0 : r0 + 512] = np.where(same, E, 0.0).sum(1)
    pos = S - np.exp(1.0 / TAU)
    neg = T - S
    return np.asarray(np.mean(np.log(neg) - np.log(pos)), dtype=np.float32)


def kernel(features, num_crops):
    feat = np.ascontiguousarray(np.asarray(features, dtype=np.float32))
    ncr = np.asarray(num_crops)
    mask = _build_mask(ncr)
    if mask is None:
        return _numpy_fallback(feat, ncr)

    _patch_compile()
    from concourse.bass_utils import run_bass_kernel_spmd

    nc = _get_program()
    in_maps, self_e = _make_inmaps(feat, mask)
    res = run_bass_kernel_spmd(nc, in_maps, core_ids=list(range(NCORES)))
    return _combine(res.results, self_e)


# revision 9
# speedup vs baseline: 1.0650x; 1.0089x over previous
"""Trainium2 Bass kernel for the contrastive loss problem.

Strategy (8 NeuronCores, SPMD, symmetric-half E):
  - Host normalizes features in f32 (exact norms), casts to bf16 and
    pre-transposes: zT [D=128, N] so the device does no transposes.
  - Global row-chunk g (of 64) is the anchor of exactly one core
    (core c anchors local chunks 0..7 = global 8c..8c+7).  Anchor a
    computes E blocks against column chunks a+1..a+32 only (each
    unordered chunk pair lands on exactly one anchor, except distance
    32 which both sides compute and count row-wise only).
  - Row sums come free from the ScalarE activation accumulator.
    Mirror (column) sums: DVE accumulates the fp16 E slabs into
    col_acc[128, 38*128]; the host does the final partition sum.
  - Diagonal blocks are bf16 like everything else; the self-term
    exp(2*||z_bf||^2) is computed exactly on the host, so the bf16
    rounding of the remaining ~3 same-group terms averages out over
    the 8192-row mean (<<1e-3 on the loss).
  - Host combines: T = own rowsums + diag + mirror colsums,
    pos = S - selfE, neg = T - S, loss = mean(log(neg) - log(pos)).
"""

import sys

import numpy as np

sys.path.insert(0, "/opt/trn_rl_repo")

N, D = 8192, 128
NCORES = 8
RPC = N // NCORES  # rows per core (1024)
CHUNKS = N // 128  # 64 chunks
ACH = 8  # anchor chunks per core
MIRC = 38  # mirror chunks with col-acc (distances 1..31 -> local 1..38)
ZTC = 40  # zt column chunks a core needs (0..39)
TAU = 0.5
EPS = 1e-8

_PROGRAM = None
_COMPILE_PATCHED = False


def _patch_compile():
    """This container's walrus build rejects two instructions that the Tile
    framework emits in its kernel tail: a Drain carrying more than one sem
    wait ("Too many sync wait commands") and the EVENT_SEMAPHORE_RANGE_CLEAR
    ISA instruction ("ISA wrong length").  Rewrite the BIR before walrus sees
    it: split multi-wait Drains into chains of single-wait Drains, and drop
    the range-clear (sems are left dirty, so one NEFF load supports a single
    execution -- kernel() is called once per process, which is our usage)."""
    global _COMPILE_PATCHED
    if _COMPILE_PATCHED:
        return
    import orjson

    import concourse.bass2jax as bass2jax
    import concourse.bass_utils as bass_utils

    orig = bass_utils.compile_bir_kernel

    def patched(bir_json, tmpdir, neff_name="file.neff"):
        bir = orjson.loads(bir_json)
        for fn in bir.get("functions", []):
            for bb in fn.get("blocks", []):
                new_insts = []
                for ins in bb.get("instructions", []):
                    if (
                        ins.get("opcode") == "ISA"
                        and ins.get("isa_opcode") == 176
                    ):
                        continue  # EVENT_SEMAPHORE_RANGE_CLEAR
                    sync = ins.get("sync_info")
                    if sync and len(sync.get("on_wait") or []) > 1:
                        waits = sync["on_wait"]
                        for k, w in enumerate(waits[:-1]):
                            pre = {
                                "engine": ins["engine"],
                                "name": f"{ins['name']}_w{k}",
                                "opcode": "Drain",
                                "ins": [],
                                "outs": [],
                                "sync_info": {"on_update": [], "on_wait": [w]},
                            }
                            if "debug" in ins:
                                pre["debug"] = ins["debug"]
                            new_insts.append(pre)
                        sync["on_wait"] = [waits[-1]]
                    new_insts.append(ins)
                bb["instructions"] = new_insts
        return orig(orjson.dumps(bir), tmpdir, neff_name=neff_name)

    bass_utils.compile_bir_kernel = patched
    bass2jax.compile_bir_kernel = patched
    _COMPILE_PATCHED = True


def _build_program():
    import concourse.bass as bass
    import concourse.mybir as mybir
    import concourse.tile as tile

    f32 = mybir.dt.float32
    f16 = mybir.dt.float16
    bf16 = mybir.dt.bfloat16
    AF = mybir.ActivationFunctionType
    AX = mybir.AxisListType

    nc = bass.Bass("TRN2", target_bir_lowering=False, debug=False)

    zt_d = nc.dram_tensor("zt", [128, ZTC * 128], bf16, kind="ExternalInput")
    mask_d = nc.dram_tensor("mask", [128, 128], f32, kind="ExternalInput")
    pout_d = nc.dram_tensor("pout", [128, 24], f32, kind="ExternalOutput")
    cacc_d = nc.dram_tensor("cacc", [128, MIRC * 128], f16, kind="ExternalOutput")

    with tile.TileContext(nc) as tc:
        with (
            tc.tile_pool(name="singles", bufs=1) as singles,
            tc.tile_pool(name="scratch", bufs=2) as scratch,
            tc.tile_pool(name="es", bufs=3) as es,
            tc.tile_pool(name="pm", bufs=2, space="PSUM") as pm,
        ):
            zt_sb = singles.tile([128, ZTC * 128], bf16)
            mask_sb = singles.tile([128, 128], f32)
            col_acc = singles.tile([128, MIRC * 128], f16)
            ediag = singles.tile([128, RPC], f32)
            tacc = singles.tile([128, 20], f32)
            pout_sb = singles.tile([128, 24], f32)

            # ---- input DMAs: first two small pieces feed the diag path
            nc.scalar.dma_start(zt_sb[:, 0:256], zt_d.ap()[:, 0:256])
            nc.sync.dma_start(zt_sb[:, 256:512], zt_d.ap()[:, 256:512])
            nc.sync.dma_start(zt_sb[:, 512:768], zt_d.ap()[:, 512:768])
            nc.scalar.dma_start(zt_sb[:, 768:1024], zt_d.ap()[:, 768:1024])
            for i in range(8):
                c0, c1 = 1024 + i * 512, 1024 + (i + 1) * 512
                eng = nc.sync if i % 2 == 0 else nc.scalar
                eng.dma_start(zt_sb[:, c0:c1], zt_d.ap()[:, c0:c1])
            nc.sync.dma_start(mask_sb[:], mask_d.ap())

            # ---- diagonal blocks (bf16; self-term fixed on host) ----
            pdiag = pm.tile([128, 2048], f32, tag="pm")
            for A in range(ACH):
                zc = zt_sb[:, A * 128 : (A + 1) * 128]
                nc.tensor.matmul(
                    pdiag[:, A * 128 : (A + 1) * 128], zc, zc,
                    start=True, stop=True,
                )
            nc.scalar.activation(
                out=ediag[:, 0:1024], in_=pdiag[:, 0:1024],
                func=AF.Exp, scale=2.0,
            )
            # tdf: per-chunk row sums of ediag
            nc.vector.reduce_sum(
                out=pout_sb[:, 8:16],
                in_=ediag.rearrange("p (a q) -> p a q", a=ACH),
                axis=AX.X,
            )
            # S: masked per-chunk sums
            for A in range(ACH):
                mtmp = scratch.tile([128, 128], f32, tag="mt")
                nc.vector.tensor_mul(
                    mtmp[:], ediag[:, A * 128 : (A + 1) * 128], mask_sb[:]
                )
                nc.vector.reduce_sum(
                    out=pout_sb[:, 16 + A : 17 + A], in_=mtmp[:], axis=AX.X
                )

            # ---- anchors: E slabs over column chunks a+1..a+32 ----
            # anchors 0..6: two [128,2048] slabs; anchor 7: four
            # [128,1024] slabs so col_acc regions finalize piecewise and
            # the output DMAs can chase them.
            def slab(a, base, width, acc_idx, add_w, copy_w):
                lhsT = zt_sb[:, a * 128 : (a + 1) * 128]
                pt = pm.tile([128, 2048], f32, tag="pm")
                for m in range(width // 512):
                    nc.tensor.matmul(
                        pt[:, m * 512 : (m + 1) * 512],
                        lhsT,
                        zt_sb[:, base + m * 512 : base + (m + 1) * 512],
                        start=True, stop=True,
                    )
                et = es.tile([128, 2048], f16)
                nc.scalar.activation(
                    out=et[:, 0:width], in_=pt[:, 0:width],
                    func=AF.Exp, scale=2.0,
                    accum_out=tacc[:, acc_idx : acc_idx + 1],
                )
                off = base - 128  # col_acc col 0 == local chunk 1
                if a == 0:
                    nc.vector.tensor_copy(
                        col_acc[:, off : off + add_w + copy_w],
                        et[:, 0 : add_w + copy_w],
                    )
                else:
                    if add_w:
                        nc.vector.tensor_add(
                            col_acc[:, off : off + add_w],
                            et[:, 0:add_w],
                            col_acc[:, off : off + add_w],
                        )
                    if copy_w:
                        nc.vector.tensor_copy(
                            col_acc[:, off + add_w : off + add_w + copy_w],
                            et[:, add_w : add_w + copy_w],
                        )

            for a in range(7):
                # s=0: chunks a+1..a+16 (all RMW for a>0); s=1: chunks
                # a+17..a+32, of which a+31 is first-touch, a+32 excluded
                slab(a, (a + 1) * 128, 2048, 2 * a, 2048, 0)
                slab(a, (a + 1) * 128 + 2048, 2048, 2 * a + 1, 1792, 128)
                if a == 6:
                    nc.sync.dma_start(
                        cacc_d.ap()[:, 0:896], col_acc[:, 0:896]
                    )

            a = 7
            for q in range(4):
                base = (a + 1) * 128 + q * 1024
                add_w = 1024 if q < 3 else 768
                copy_w = 128 if q == 3 else 0
                slab(a, base, 1024, 14 + q, add_w, copy_w)
                # chase: this slab finalized col_acc [896+1024q, ...).
                # Small pieces on idle queues (not scalar: it runs exp).
                p0 = 896 + q * 1024
                if q < 3:
                    nc.sync.dma_start(
                        cacc_d.ap()[:, p0 : p0 + 512], col_acc[:, p0 : p0 + 512]
                    )
                    nc.gpsimd.dma_start(
                        cacc_d.ap()[:, p0 + 512 : p0 + 1024],
                        col_acc[:, p0 + 512 : p0 + 1024],
                    )
                else:
                    for k in range(4):
                        eng = nc.sync if k % 2 == 0 else nc.gpsimd
                        s0 = p0 + k * 224
                        s1 = min(s0 + 224, 4864)
                        eng.dma_start(
                            cacc_d.ap()[:, s0:s1], col_acc[:, s0:s1]
                        )

            # T_main = sum of the per-anchor slab accumulators
            nc.vector.reduce_sum(
                out=pout_sb[:, 0:7],
                in_=tacc[:, 0:14].rearrange("p (a s) -> p a s", a=7),
                axis=AX.X,
            )
            nc.vector.reduce_sum(
                out=pout_sb[:, 7:8],
                in_=tacc[:, 14:18].rearrange("p (a s) -> p a s", a=1),
                axis=AX.X,
            )

            # ---- outputs ----
            nc.sync.dma_start(pout_d.ap(), pout_sb[:])

    return nc


def _get_program():
    global _PROGRAM
    if _PROGRAM is None:
        _PROGRAM = _build_program()
    return _PROGRAM


def _group_ids(num_crops):
    ids = np.repeat(np.arange(num_crops.shape[0], dtype=np.int64), num_crops)
    if ids.shape[0] >= N:
        return ids[:N]
    return np.pad(ids, (0, N - ids.shape[0]), mode="edge")


def _build_mask(num_crops):
    """[128,128] same-group mask, valid when the group pattern repeats
    every 128 rows and no group straddles a 128-row boundary."""
    ids = _group_ids(num_crops)
    pat = ids.reshape(CHUNKS, 128)
    local = pat - pat[:, :1]
    if not (local == local[0]).all():
        return None
    if (pat[1:, 0] <= pat[:-1, -1]).any():
        return None
    return (local[0][:, None] == local[0][None, :]).astype(np.float32)


def _prep(feat):
    """Host prep: exact f32 normalize, bf16 cast, transpose, self-terms."""
    import ml_dtypes

    nrm = np.maximum(np.sqrt((feat.astype(np.float64) ** 2).sum(-1)), EPS)
    z32 = (feat / nrm[:, None]).astype(np.float32)
    zbf = z32.astype(ml_dtypes.bfloat16)
    zbfT = np.ascontiguousarray(zbf.T)  # [128, N]
    self_e = np.exp(2.0 * (zbf.astype(np.float64) ** 2).sum(-1))  # [N]
    return zbfT, self_e


def _make_inmaps(feat, mask):
    zbfT, self_e = _prep(feat)
    in_maps = []
    for c in range(NCORES):
        zt = np.ascontiguousarray(
            np.roll(zbfT, -RPC * c, axis=1)[:, : ZTC * 128]
        )
        in_maps.append({"zt": zt, "mask": mask})
    return in_maps, self_e


def _combine(results, self_e):
    """Host combine of per-core partials -> loss (f64)."""
    T = np.zeros(N, np.float64)
    S = np.zeros(N, np.float64)
    for c in range(NCORES):
        r = results[c]
        pout = r["pout"].astype(np.float64)  # [128, 24]
        tmain, tdf, s = pout[:, 0:8], pout[:, 8:16], pout[:, 16:24]
        for a in range(ACH):
            g = 8 * c + a
            rows = slice(g * 128, (g + 1) * 128)
            T[rows] += tmain[:, a] + tdf[:, a]
            S[rows] = s[:, a]
        cs = r["cacc"].astype(np.float64).sum(axis=0).reshape(MIRC, 128)
        for j in range(1, MIRC + 1):
            g = (8 * c + j) % CHUNKS
            T[g * 128 : (g + 1) * 128] += cs[j - 1]
    pos = S - self_e
    neg = T - S
    return np.asarray(np.mean(np.log(neg) - np.log(pos)), dtype=np.float32)


def _numpy_fallback(feat, num_crops):
    ids = _group_ids(num_crops)
    nrm = np.maximum(np.sqrt((feat.astype(np.float64) ** 2).sum(-1)), EPS)
    z = feat / nrm[:, None].astype(np.float32)
    T = np.empty(N, np.float64)
    S = np.empty(N, np.float64)
    for r0 in range(0, N, 512):
        E = np.exp((z[r0 : r0 + 512] @ z.T) / TAU).astype(np.float64)
        same = ids[r0 : r0 + 512, None] == ids[None, :]
        T[r0 : r0 + 512] = E.sum(1)
        S[r0 : r0 + 512] = np.where(same, E, 0.0).sum(1)
    pos = S - np.exp(1.0 / TAU)
    neg = T - S
    return np.asarray(np.mean(np.log(neg) - np.log(pos)), dtype=np.float32)


def kernel(features, num_crops):
    feat = np.ascontiguousarray(np.asarray(features, dtype=np.float32))
    ncr = np.asarray(num_crops)
    mask = _build_mask(ncr)
    if mask is None:
        return _numpy_fallback(feat, ncr)

    _patch_compile()
    from concourse.bass_utils import run_bass_kernel_spmd

    nc = _get_program()
    in_maps, self_e = _make_inmaps(feat, mask)
    res = run_bass_kernel_spmd(nc, in_maps, core_ids=list(range(NCORES)))
    return _combine(res.results, self_e)
